# revision 17
# baseline (speedup 1.0000x reference)
"""VQ codebook assignment (ApplyKmeans) on 8 Trainium2 NeuronCores.

tokens[n] = argmin_k ||x_n - c_k||^2 = argmax_k (x_n.c_k - Cnorm_k/2)
(||x_n||^2 is constant per row). Data-parallel: x sharded along N
across 8 cores, C/Cnorm replicated.

Per core (16384 rows, 128 row-tiles of 128 rows), v8 schedule:
single-bank PSUM tiles + PSUM-direct argmax (measured best-in-class
tail) + arrival-order-matched startup:
  - host pre-tiles x^T so each [128d, 128n] stationary tile is
    contiguous (fp16: halves HBM traffic; PSUM accumulates fp32;
    52/131072 argmin flips vs the fp32 reference, rel err 0.0144)
  - per tile: 8 accumulating matmuls (x^T chunk stationary, C chunk
    moving, ~128.3ns/MM steady) on top of a bias pre-load, so a
    [128, 300] PSUM tile (8 cycling banks) holds val = x.C - Cnorm/2
  - bias pre-load: tiles 0-7 matmul the bias in (ones x [-Cnorm/2
    hi/lo] rows, start=True), which also covers the PSUM has_written
    bits; tiles 8+ get a ScalarE ACTIVATE bias write emitted one tile
    ahead of the PE (~543ns, 53% of the scalar queue)
  - DVE max8 + max_index run directly on PSUM (478+469+91ns/tile,
    fractionally above the 1028ns/tile PE budget -- the ~10ns/tile
    deficit is absorbed by the 8-bank decoupling; an SBUF-evict
    variant measured ~1.2us MORE tail from the eviction latency, and
    2-bank pair tiles cost ~1us more tail from coarse dep granularity)
  - startup (measured: ~7.6us framework preamble, first DMA data
    ~9.0us, early HBM ramps 200->420GB/s and the ~7MB front is
    bandwidth-critical): DMA_DIRECT2D issues cost ~670ns each on the
    in-order queues and completion sems fire per whole transfer, so
    groups 0-2 go out as 4 two-chunk batches each and cons as
    [bias+ones+c0+c1 | c2-4 | c5-7] on the parallel scalar ring;
    tiles 0-11 run j-major (all 4 tiles' matmuls per chunk
    back-to-back) so each arriving batch unlocks ~1us of PE work;
    groups 3+ are monolithic, pre-issued 3 deep and prefetched 3
    groups ahead
  - 12 dep-free warmup matmuls over a memset tile bridge the PE from
    7.6us into the first real matmuls so the HAM activity window
    never sees a >3.4us idle gap (PE warms at ~11.5us instead of
    ~19us; cold matmuls run at 1.2GHz)
  - the fp32 bias table is built on-chip (one ones x [bh; bl] matmul
    broadcast + one startup ACTIVATE eviction) instead of a 307KB DMA
    on the bandwidth-critical front
  - a tiny early ACTIVATE hoists the ~1.3us ACT_TABLE_LOAD into the
    startup DMA wait
  - tokens compacted on the otherwise-idle Pool engine and streamed
    out in 16-tile blocks through t=112, then 8/7/1; flush DMA issues
    ride the idle sync queue, deferred one tile so they never park
    ahead of an x prefetch issue; the last flush carries a single
    tile so only ~128 4B packets remain after the final argmax

Row interleaving: row-tile t holds rows {p*128 + t}, so the token
buffer [p, t] DMAs out contiguously in original row order.

Walrus only lowers one sync wait per instruction; _hoist_excess_waits
moves Tile's extra waits onto same-engine no-ops at the same program
point. x loads share the sync HWDGE ring (same-ring transfers complete
in order, so prefetch can't starve urgent loads); constants ride the
scalar ring.

Profiling notes: engine clock varies run to run (2.4 vs ~2.0 GHz
throttle episodes: check MATMUL median duration ~290ns vs ~348ns
before comparing timings; full-clock run-to-run sigma is ~1.3us).
Structural budget at full clock: ~7.6us preamble + ~1.5us first-data
latency + ~132us PE work + ~5us front HBM-ramp deficit (all 8 cores
pull the ~7MB front through shared HBM simultaneously) + ~5us tail
(DVE chain + token store + sem drain) => ~154us plateau.
"""
import os
import sys

import numpy as np

if "/opt/trn_rl_repo" not in sys.path:
    sys.path.insert(0, "/opt/trn_rl_repo")

import concourse.bass as bass
import concourse.mybir as mybir
import concourse.tile_sem_assignment as _tsa
from concourse.bass_utils import run_bass_kernel_spmd
from concourse.tile import TileContext

_tsa.NUM_HWDGE_SEMS = int(os.environ.get("KM_HW_LANES", "8"))

_orig_assign_tick = _tsa.TileClockTick._assign_tick


def _assign_tick_lanepools(self, inst):
    try:
        if isinstance(inst, _tsa.DMAInst) and inst.engine != mybir.EngineType.Pool:
            if not hasattr(self, "_lane_ctr"):
                self._lane_ctr = {}
            eng = inst.engine
            n = _tsa.NUM_HWDGE_SEMS
            half = max(1, n // 2)
            pool = (
                list(range(0, half))
                if eng == mybir.EngineType.Activation
                else list(range(half, n))
            )
            c = self._lane_ctr.get(eng, 0)
            self.next_hw_dma_idx = pool[c % len(pool)]
            self._lane_ctr[eng] = c + 1
    except Exception:
        pass
    return _orig_assign_tick(self, inst)


_tsa.TileClockTick._assign_tick = _assign_tick_lanepools

P = 128
D = 1024
K = 300
NCORES = 8
ROWS = 16384
TILES = ROWS // P
GROUPS = 32
TPG = TILES // GROUPS
DCH = D // P
CB = K + P  # offset of c0 within the cons row
JM_GROUPS = 3  # groups 0..2 (tiles 0-11) run j-major on batched loads

F16 = mybir.dt.float16
F32 = mybir.dt.float32
I32 = mybir.dt.int32
U32 = mybir.dt.uint32

LAST_RESULT = None


def _ensure_ntff_hook():
    try:
        from antenv.axon_hooks import get_axon_ntff_profile_hook  # noqa: F401

        return
    except ImportError:
        pass
    import types

    import antenv

    try:
        from trn_agent_boot.trn_boot import _ntff_profile_via_ctypes
    except ImportError:
        return
    mod = types.ModuleType("antenv.axon_hooks")
    _hook = [None]
    mod.set_axon_ntff_profile_hook = lambda h: _hook.__setitem__(0, h)
    mod.get_axon_ntff_profile_hook = lambda: _hook[0]
    sys.modules["antenv.axon_hooks"] = mod
    antenv.axon_hooks = mod
    so = "/opt/axon/libaxon_pjrt.so"
    if os.path.exists(so):
        mod.set_axon_ntff_profile_hook(_ntff_profile_via_ctypes(so))


def build_nc() -> bass.Bass:
    nc = bass.Bass()

    xg = nc.declare_dram_parameter("xg", [GROUPS, P, DCH * TPG * P], F16, isOutput=False)
    cons = nc.declare_dram_parameter("cons", [P, DCH * K + K + P], F16, isOutput=False)
    out = nc.declare_dram_parameter("out", [P, TILES], I32, isOutput=True)

    FLUSH = [16, 32, 48, 64, 80, 96, 112, 120, 127, 128]

    with TileContext(nc) as tc:
        with (
            tc.tile_pool(name="const", bufs=1) as constp,
            tc.tile_pool(name="warm", bufs=1) as warmp,
            tc.tile_pool(name="xp0", bufs=4 * JM_GROUPS) as xp0,
            tc.tile_pool(name="xp", bufs=4) as xp,
            tc.tile_pool(name="mx", bufs=8) as mxp,
            tc.tile_pool(name="psum", bufs=8, space="PSUM") as psp,
            tc.tile_pool(name="outp", bufs=1) as outp,
        ):
            # cons host layout [bias K | ones P | c0..c7], split
            # [bias+ones+c0 | c1+c2 | c3+c4 | c5-7]: completion sems
            # fire ~1.5us after a transfer's last byte, so the first
            # piece carries exactly what the bias matmuls and the
            # j-major chunk-0 matmuls need and nothing more
            cons_t = constp.tile([P, DCH * K + K + P], F16)
            nc.scalar.dma_start(out=cons_t[:, : CB + K], in_=cons[:, : CB + K])
            nc.scalar.dma_start(
                out=cons_t[:, CB + K : CB + 3 * K],
                in_=cons[:, CB + K : CB + 3 * K],
            )
            nc.scalar.dma_start(
                out=cons_t[:, CB + 3 * K : CB + 5 * K],
                in_=cons[:, CB + 3 * K : CB + 5 * K],
            )
            nc.scalar.dma_start(
                out=cons_t[:, CB + 5 * K :], in_=cons[:, CB + 5 * K :]
            )
            ctiles = [
                cons_t[:, CB + j * K : CB + (j + 1) * K] for j in range(DCH)
            ]
            btile = cons_t[:, :K]
            otile = cons_t[:, K:CB]
            bft = constp.tile([P, K], F32)

            # PE warmup: dep-free matmuls over a memset tile. They
            # bridge the PE from the preamble into the first real
            # matmuls so HAM never sees a >3.4us idle window, and
            # their start=True writes cover the PSUM has_written bits
            # alongside the tiles-0-7 bias matmuls.
            warm = warmp.tile([P, K], F16)
            nc.gpsimd.memset(warm[:], 0.0)
            # tiny dummy ACTIVATE: hoists the ~1.3us ACT_TABLE_LOAD
            # into the startup DMA wait
            wsc = warmp.tile([P, 8], F32, name="wsc")
            nc.scalar.copy(out=wsc[:1, :], in_=warm[:1, :8])
            for w in range(15):
                wps = psp.tile([P, K], F32, name="ps")
                nc.tensor.matmul(
                    wps[:], lhsT=warm[:, :P], rhs=warm[:],
                    start=True, stop=True,
                )
            # fp32 bias table built on-chip instead of a 307KB DMA on
            # the bandwidth-critical front: ones x [bh; bl] broadcast
            # into a PSUM scratch (fp32-exact bh+bl), evicted once
            bias_ps = psp.tile([P, K], F32, name="ps")
            nc.tensor.matmul(
                bias_ps[:], lhsT=otile[:], rhs=btile[:], start=True, stop=True
            )
            nc.scalar.copy(out=bft[:], in_=bias_ps[:])

            # x loads: groups 0-2 in 4 two-chunk batches each (few
            # ~670ns issue slots, progressive completion sems on the
            # in-order ring); groups 3+ monolithic, pre-issued deep
            xbat = {}
            gbufs = {}

            def issue_group_batched(g):
                bufs = []
                for i in range(4):
                    cb = xp0.tile([P, 2, TPG, P], F16, name="xb")
                    nc.sync.dma_start(
                        out=cb[:],
                        in_=xg[
                            g, :, 2 * i * TPG * P : 2 * (i + 1) * TPG * P
                        ].rearrange("p (j t q) -> p j t q", j=2, t=TPG),
                    )
                    bufs.append(cb)
                xbat[g] = bufs

            def issue_group(g):
                xbuf = xp.tile([P, DCH, TPG, P], F16, name="xgrp")
                nc.sync.dma_start(
                    out=xbuf[:],
                    in_=xg[g].rearrange("p (j t q) -> p j t q", j=DCH, t=TPG),
                )
                gbufs[g] = xbuf

            for g in range(JM_GROUPS):
                issue_group_batched(g)
            issue_group(3)
            issue_group(4)
            issue_group(5)

            def chunk_of(g):
                if g < JM_GROUPS:
                    return lambda j, tl, bufs=xbat[g]: bufs[j // 2][:, j % 2, tl, :]
                return lambda j, tl, xb=gbufs[g]: xb[:, j, tl, :]

            idxbuf = outp.tile([P, TILES, 8], U32)
            tokbuf = outp.tile([P, TILES], I32)

            def emit_flush(lo, hi):
                # token compaction on the otherwise-idle Pool engine;
                # the DMA issue rides the (idle) sync queue
                nc.gpsimd.tensor_copy(
                    out=tokbuf[:, lo:hi], in_=idxbuf[:, lo:hi, 0]
                )
                nc.sync.dma_start(out=out[:, lo:hi], in_=tokbuf[:, lo:hi])

            def dve_tile(t, values):
                mx = mxp.tile([P, 8], F32)
                nc.vector.max(out=mx[:], in_=values)
                nc.vector.max_index(
                    out=idxbuf[:, t, :], in_max=mx[:], in_values=values
                )

            ps_tiles = {}

            def alloc_ps(t):
                ps_tiles[t] = psp.tile([P, K], F32, name="ps")
                return ps_tiles[t]

            def bias_write(t):
                nc.scalar.copy(out=alloc_ps(t)[:], in_=bft[:])

            # ---- prologue: tiles 0-11 (groups 0-2) j-major so every
            # arriving 2-chunk batch unlocks 4 tiles' worth of matmuls;
            # each tile's argmax is emitted right after its last matmul
            for g in range(JM_GROUPS):
                t0 = 4 * g
                chunk = chunk_of(g)
                if g == 2:
                    # scalar-bias path for tiles 8-12: emitted before
                    # this group's matmuls; each write's WAR (the DVE
                    # reads of the banks 8 tiles back) is already done
                    for t in range(8, 13):
                        bias_write(t)
                else:
                    for tl in range(4):
                        nc.tensor.matmul(
                            alloc_ps(t0 + tl)[:],
                            lhsT=otile[:], rhs=btile[:],
                            start=True, stop=False,
                            skip_group_check=True,
                        )
                for j in range(DCH):
                    last = j == DCH - 1
                    for tl in range(4):
                        nc.tensor.matmul(
                            ps_tiles[t0 + tl][:],
                            lhsT=chunk(j, tl), rhs=ctiles[j][:],
                            start=False, stop=last,
                            skip_group_check=True,
                        )
                        if last:
                            dve_tile(t0 + tl, ps_tiles[t0 + tl][:])

            # ---- tiles 12-127: steady state, one tile per iteration
            pending = None
            for t in range(4 * JM_GROUPS, TILES):
                g = t // TPG
                chunk = chunk_of(g)
                if t % TPG == 0 and g + 3 < GROUPS and (g + 3) not in gbufs:
                    # prefetch three groups ahead at each group start
                    # (xp bufs=4: active + 3 in flight)
                    issue_group(g + 3)
                ps = ps_tiles[t]
                if t + 2 < TILES + 1 and (t + 1) > 12:
                    # bias write for the NEXT tile, one tile ahead of
                    # the PE so the in-order scalar queue never stalls
                    # the matmul stream (t=12's was emitted in the
                    # prologue block)
                    bias_write(t + 1)
                # deferred flush: emitted after the next tile's bias
                # copy so the token-DMA issue never parks ahead of an
                # x prefetch issue on the sync queue
                if pending is not None:
                    emit_flush(*pending)
                    pending = None

                for j in range(DCH):
                    nc.tensor.matmul(
                        ps[:],
                        lhsT=chunk(j, t % TPG),
                        rhs=ctiles[j][:],
                        start=False,
                        stop=(j == DCH - 1),
                        skip_group_check=True,
                    )
                dve_tile(t, ps[:])
                if (t + 1) in FLUSH:
                    s = FLUSH[FLUSH.index(t + 1) - 1] if (t + 1) != FLUSH[0] else 0
                    if t + 1 == TILES:
                        # final 1-tile store reads idxbuf directly
                        # (u32 index bits == i32 token), skipping the
                        # Pool CAST hop after the last match
                        nc.sync.dma_start(
                            out=out[:, t : t + 1],
                            in_=idxbuf[:, t, 0:1].bitcast(I32),
                        )
                    elif t + 1 == TILES - 1:
                        # the 7-tile store overlaps tile 127's argmax
                        emit_flush(s, t + 1)
                    else:
                        pending = (s, t + 1)

    _hoist_excess_waits(nc)
    return nc


def _hoist_excess_waits(nc: bass.Bass, max_waits: int = 1):
    n = 0
    for f in nc.m.functions:
        for blk in f.blocks:
            insts = blk.instructions
            i = 0
            while i < len(insts):
                inst = insts[i]
                si = inst.sync_info
                if si and si.on_wait and len(si.on_wait) > max_waits:
                    waits = list(si.on_wait)
                    si.on_wait = waits[-max_waits:]
                    inst.sync_info = si
                    pre = []
                    for j in range(0, len(waits) - max_waits, max_waits):
                        nd = mybir.InstNoOp(name=f"I-wsplit{n}", ins=[], outs=[])
                        n += 1
                        nd.engine = inst.engine
                        nsi = type(si)(
                            on_wait=waits[j : j + max_waits], on_update=[]
                        )
                        nd.sync_info = nsi
                        try:
                            nc.register_instruction(nd, overwrite=True)
                        except Exception:
                            pass
                        pre.append(nd)
                    for k, nd in enumerate(pre):
                        insts.insert(i + k, nd)
                    i += len(pre)
                i += 1


def make_in_maps(x, C, Cnorm):
    x16 = x.astype(np.float16)
    C16 = C.astype(np.float16).reshape(DCH, P, K)

    bz = (-0.5 * Cnorm.reshape(K)).astype(np.float32)
    bh = bz.astype(np.float16)
    bl = (bz - bh.astype(np.float32)).astype(np.float16)

    cons = np.zeros((P, DCH * K + K + P), np.float16)
    cons[0, :K] = bh
    cons[1, :K] = bl
    cons[0:2, K : K + P] = 1.0
    cons[:, K + P :] = C16.transpose(1, 0, 2).reshape(P, DCH * K)

    in_maps = []
    for c in range(NCORES):
        xs = x16[c * ROWS : (c + 1) * ROWS]
        xr = xs.reshape(P, GROUPS, TPG, DCH, P)
        xgc = np.ascontiguousarray(xr.transpose(1, 4, 3, 2, 0))
        in_maps.append(
            {
                "xg": xgc.reshape(GROUPS, P, DCH * TPG * P),
                "cons": cons,
            }
        )
    return in_maps


_NC_CACHE = {}


def kernel(x, C, Cnorm, b, t):
    global LAST_RESULT
    x = np.asarray(x)
    C = np.asarray(C)
    Cnorm = np.asarray(Cnorm)

    key = 0
    if key not in _NC_CACHE:
        _NC_CACHE[key] = build_nc()
    nc = _NC_CACHE[key]

    in_maps = make_in_maps(x, C, Cnorm)
    trace = bool(int(os.environ.get("KM_TRACE", "0")))
    if trace:
        _ensure_ntff_hook()
    res = run_bass_kernel_spmd(
        nc, in_maps, core_ids=list(range(NCORES)), trace=trace
    )
    LAST_RESULT = res

    shards = [res.results[c]["out"].reshape(-1) for c in range(NCORES)]
    tokens = np.concatenate(shards).astype(np.int32)
    return tokens.reshape(int(b), int(t))


# revision 19
# speedup vs baseline: 1.1588x; 1.1588x over previous
"""VQ codebook assignment (ApplyKmeans) on 8 Trainium2 NeuronCores.

tokens[n] = argmin_k ||x_n - c_k||^2 = argmax_k (x_n.c_k - Cnorm_k/2)
(||x_n||^2 is constant per row). Data-parallel: x sharded along N
across 8 cores, C/Cnorm replicated.

Per core (16384 rows, 128 row-tiles of 128 rows), v8 schedule:
single-bank PSUM tiles + PSUM-direct argmax (measured best-in-class
tail) + arrival-order-matched startup:
  - host pre-tiles x^T so each [128d, 128n] stationary tile is
    contiguous (fp16: halves HBM traffic; PSUM accumulates fp32;
    52/131072 argmin flips vs the fp32 reference, rel err 0.0144)
  - per tile: 8 accumulating matmuls (x^T chunk stationary, C chunk
    moving, ~128.3ns/MM steady) on top of a bias pre-load, so a
    [128, 300] PSUM tile (8 cycling banks) holds val = x.C - Cnorm/2
  - bias pre-load: tiles 0-7 matmul the bias in (ones x [-Cnorm/2
    hi/lo] rows, start=True), which also covers the PSUM has_written
    bits; tiles 8+ get a ScalarE ACTIVATE bias write emitted one tile
    ahead of the PE (~543ns, 53% of the scalar queue)
  - DVE max8 + max_index run directly on PSUM (478+469+91ns/tile,
    fractionally above the 1028ns/tile PE budget -- the ~10ns/tile
    deficit is absorbed by the 8-bank decoupling; an SBUF-evict
    variant measured ~1.2us MORE tail from the eviction latency, and
    2-bank pair tiles cost ~1us more tail from coarse dep granularity)
  - startup (measured: ~7.6us framework preamble, first DMA data
    ~9.0us, early HBM ramps 200->420GB/s and the ~7MB front is
    bandwidth-critical): DMA_DIRECT2D issues cost ~670ns each on the
    in-order queues and completion sems fire per whole transfer, so
    groups 0-2 go out as 4 two-chunk batches each and cons as
    [bias+ones+c0+c1 | c2-4 | c5-7] on the parallel scalar ring;
    tiles 0-11 run j-major (all 4 tiles' matmuls per chunk
    back-to-back) so each arriving batch unlocks ~1us of PE work;
    groups 3+ are monolithic, pre-issued 3 deep and prefetched 3
    groups ahead
  - 12 dep-free warmup matmuls over a memset tile bridge the PE from
    7.6us into the first real matmuls so the HAM activity window
    never sees a >3.4us idle gap (PE warms at ~11.5us instead of
    ~19us; cold matmuls run at 1.2GHz)
  - the fp32 bias table is built on-chip (one ones x [bh; bl] matmul
    broadcast + one startup ACTIVATE eviction) instead of a 307KB DMA
    on the bandwidth-critical front
  - a tiny early ACTIVATE hoists the ~1.3us ACT_TABLE_LOAD into the
    startup DMA wait
  - tokens compacted on the otherwise-idle Pool engine and streamed
    out in 16-tile blocks through t=112, then 8/7/1; flush DMA issues
    ride the idle sync queue, deferred one tile so they never park
    ahead of an x prefetch issue; the last flush carries a single
    tile so only ~128 4B packets remain after the final argmax

Row interleaving: row-tile t holds rows {p*128 + t}, so the token
buffer [p, t] DMAs out contiguously in original row order.

Walrus only lowers one sync wait per instruction; _hoist_excess_waits
moves Tile's extra waits onto same-engine no-ops at the same program
point. x loads share the sync HWDGE ring (same-ring transfers complete
in order, so prefetch can't starve urgent loads); constants ride the
scalar ring.

Profiling notes: engine clock varies run to run (2.4 vs ~2.0 GHz
throttle episodes: check MATMUL median duration ~290ns vs ~348ns
before comparing timings; full-clock run-to-run sigma is ~1.3us).
Structural budget at full clock: ~7.6us preamble + ~1.5us first-data
latency + ~132us PE work + ~5us front HBM-ramp deficit (all 8 cores
pull the ~7MB front through shared HBM simultaneously) + ~5us tail
(DVE chain + token store + sem drain) => ~154us plateau.
"""
import os
import sys

import numpy as np

if "/opt/trn_rl_repo" not in sys.path:
    sys.path.insert(0, "/opt/trn_rl_repo")

import concourse.bass as bass
import concourse.mybir as mybir
import concourse.tile_sem_assignment as _tsa
from concourse.bass_utils import run_bass_kernel_spmd
from concourse.tile import TileContext

_tsa.NUM_HWDGE_SEMS = int(os.environ.get("KM_HW_LANES", "8"))

_orig_assign_tick = _tsa.TileClockTick._assign_tick


def _assign_tick_lanepools(self, inst):
    try:
        if isinstance(inst, _tsa.DMAInst) and inst.engine != mybir.EngineType.Pool:
            if not hasattr(self, "_lane_ctr"):
                self._lane_ctr = {}
            eng = inst.engine
            n = _tsa.NUM_HWDGE_SEMS
            half = max(1, n // 2)
            pool = (
                list(range(0, half))
                if eng == mybir.EngineType.Activation
                else list(range(half, n))
            )
            c = self._lane_ctr.get(eng, 0)
            self.next_hw_dma_idx = pool[c % len(pool)]
            self._lane_ctr[eng] = c + 1
    except Exception:
        pass
    return _orig_assign_tick(self, inst)


_tsa.TileClockTick._assign_tick = _assign_tick_lanepools

P = 128
D = 1024
K = 300
NCORES = 8
ROWS = 16384
TILES = ROWS // P
GROUPS = 32
TPG = TILES // GROUPS
DCH = D // P
CB = K + P  # offset of c0 within the cons row
JM_GROUPS = 3  # groups 0..2 (tiles 0-11) run j-major on batched loads

F16 = mybir.dt.float16
F32 = mybir.dt.float32
I32 = mybir.dt.int32
U32 = mybir.dt.uint32

LAST_RESULT = None


def _ensure_ntff_hook():
    try:
        from antenv.axon_hooks import get_axon_ntff_profile_hook  # noqa: F401

        return
    except ImportError:
        pass
    import types

    import antenv

    try:
        from trn_agent_boot.trn_boot import _ntff_profile_via_ctypes
    except ImportError:
        return
    mod = types.ModuleType("antenv.axon_hooks")
    _hook = [None]
    mod.set_axon_ntff_profile_hook = lambda h: _hook.__setitem__(0, h)
    mod.get_axon_ntff_profile_hook = lambda: _hook[0]
    sys.modules["antenv.axon_hooks"] = mod
    antenv.axon_hooks = mod
    so = "/opt/axon/libaxon_pjrt.so"
    if os.path.exists(so):
        mod.set_axon_ntff_profile_hook(_ntff_profile_via_ctypes(so))


def build_nc() -> bass.Bass:
    nc = bass.Bass()

    xg = nc.declare_dram_parameter("xg", [GROUPS, P, DCH * TPG * P], F16, isOutput=False)
    cons = nc.declare_dram_parameter("cons", [P, DCH * K + K + P], F16, isOutput=False)
    out = nc.declare_dram_parameter("out", [P, TILES], I32, isOutput=True)

    FLUSH = [16, 32, 48, 64, 80, 96, 112, 120, 127, 128]

    with TileContext(nc) as tc:
        with (
            tc.tile_pool(name="const", bufs=1) as constp,
            tc.tile_pool(name="warm", bufs=1) as warmp,
            tc.tile_pool(name="xp0", bufs=4 * JM_GROUPS) as xp0,
            tc.tile_pool(name="xp", bufs=4) as xp,
            tc.tile_pool(name="mx", bufs=8) as mxp,
            tc.tile_pool(name="psum", bufs=8, space="PSUM") as psp,
            tc.tile_pool(name="outp", bufs=1) as outp,
        ):
            # cons host layout [bias K | ones P | c0..c7], split
            # [bias+ones+c0 | c1+c2 | c3+c4 | c5-7]: completion sems
            # fire ~1.5us after a transfer's last byte, so the first
            # piece carries exactly what the bias matmuls and the
            # j-major chunk-0 matmuls need and nothing more
            cons_t = constp.tile([P, DCH * K + K + P], F16)
            nc.scalar.dma_start(out=cons_t[:, : CB + K], in_=cons[:, : CB + K])
            nc.scalar.dma_start(
                out=cons_t[:, CB + K : CB + 3 * K],
                in_=cons[:, CB + K : CB + 3 * K],
            )
            nc.scalar.dma_start(
                out=cons_t[:, CB + 3 * K : CB + 5 * K],
                in_=cons[:, CB + 3 * K : CB + 5 * K],
            )
            nc.scalar.dma_start(
                out=cons_t[:, CB + 5 * K :], in_=cons[:, CB + 5 * K :]
            )
            ctiles = [
                cons_t[:, CB + j * K : CB + (j + 1) * K] for j in range(DCH)
            ]
            btile = cons_t[:, :K]
            otile = cons_t[:, K:CB]
            bft = constp.tile([P, K], F32)

            # PE warmup: dep-free matmuls over a memset tile. They
            # bridge the PE from the preamble into the first real
            # matmuls so HAM never sees a >3.4us idle window, and
            # their start=True writes cover the PSUM has_written bits
            # alongside the tiles-0-7 bias matmuls.
            warm = warmp.tile([P, K], F16)
            nc.gpsimd.memset(warm[:], 0.0)
            # tiny dummy ACTIVATE: hoists the ~1.3us ACT_TABLE_LOAD
            # into the startup DMA wait
            wsc = warmp.tile([P, 8], F32, name="wsc")
            nc.scalar.copy(out=wsc[:1, :], in_=warm[:1, :8])
            for w in range(15):
                wps = psp.tile([P, K], F32, name="ps")
                nc.tensor.matmul(
                    wps[:], lhsT=warm[:, :P], rhs=warm[:],
                    start=True, stop=True,
                )
            # fp32 bias table built on-chip instead of a 307KB DMA on
            # the bandwidth-critical front: ones x [bh; bl] broadcast
            # into a PSUM scratch (fp32-exact bh+bl), evicted once
            bias_ps = psp.tile([P, K], F32, name="ps")
            nc.tensor.matmul(
                bias_ps[:], lhsT=otile[:], rhs=btile[:], start=True, stop=True
            )
            nc.scalar.copy(out=bft[:], in_=bias_ps[:])

            # x loads: groups 0-2 in 4 two-chunk batches each (few
            # ~670ns issue slots, progressive completion sems on the
            # in-order ring); groups 3+ monolithic, pre-issued deep
            xbat = {}
            gbufs = {}

            def issue_group_batched(g):
                bufs = []
                for i in range(4):
                    cb = xp0.tile([P, 2, TPG, P], F16, name="xb")
                    nc.sync.dma_start(
                        out=cb[:],
                        in_=xg[
                            g, :, 2 * i * TPG * P : 2 * (i + 1) * TPG * P
                        ].rearrange("p (j t q) -> p j t q", j=2, t=TPG),
                    )
                    bufs.append(cb)
                xbat[g] = bufs

            def issue_group(g):
                xbuf = xp.tile([P, DCH, TPG, P], F16, name="xgrp")
                nc.sync.dma_start(
                    out=xbuf[:],
                    in_=xg[g].rearrange("p (j t q) -> p j t q", j=DCH, t=TPG),
                )
                gbufs[g] = xbuf

            for g in range(JM_GROUPS):
                issue_group_batched(g)
            issue_group(3)
            issue_group(4)
            issue_group(5)

            def chunk_of(g):
                if g < JM_GROUPS:
                    return lambda j, tl, bufs=xbat[g]: bufs[j // 2][:, j % 2, tl, :]
                return lambda j, tl, xb=gbufs[g]: xb[:, j, tl, :]

            idxbuf = outp.tile([P, TILES, 8], U32)
            tokbuf = outp.tile([P, TILES], I32)

            def emit_flush(lo, hi):
                # token compaction on the otherwise-idle Pool engine;
                # the DMA issue rides the (idle) sync queue
                nc.gpsimd.tensor_copy(
                    out=tokbuf[:, lo:hi], in_=idxbuf[:, lo:hi, 0]
                )
                nc.sync.dma_start(out=out[:, lo:hi], in_=tokbuf[:, lo:hi])

            def dve_tile(t, values):
                mx = mxp.tile([P, 8], F32)
                nc.vector.max(out=mx[:], in_=values)
                nc.vector.max_index(
                    out=idxbuf[:, t, :], in_max=mx[:], in_values=values
                )

            ps_tiles = {}

            def alloc_ps(t):
                ps_tiles[t] = psp.tile([P, K], F32, name="ps")
                return ps_tiles[t]

            def bias_write(t):
                nc.scalar.copy(out=alloc_ps(t)[:], in_=bft[:])

            # ---- prologue: tiles 0-11 (groups 0-2) j-major so every
            # arriving 2-chunk batch unlocks 4 tiles' worth of matmuls;
            # each tile's argmax is emitted right after its last matmul
            for g in range(JM_GROUPS):
                t0 = 4 * g
                chunk = chunk_of(g)
                if g == 2:
                    # scalar-bias path for tiles 8-12: emitted before
                    # this group's matmuls; each write's WAR (the DVE
                    # reads of the banks 8 tiles back) is already done
                    for t in range(8, 13):
                        bias_write(t)
                else:
                    for tl in range(4):
                        nc.tensor.matmul(
                            alloc_ps(t0 + tl)[:],
                            lhsT=otile[:], rhs=btile[:],
                            start=True, stop=False,
                            skip_group_check=True,
                        )
                for j in range(DCH):
                    last = j == DCH - 1
                    for tl in range(4):
                        nc.tensor.matmul(
                            ps_tiles[t0 + tl][:],
                            lhsT=chunk(j, tl), rhs=ctiles[j][:],
                            start=False, stop=last,
                            skip_group_check=True,
                        )
                        if last:
                            dve_tile(t0 + tl, ps_tiles[t0 + tl][:])

            # ---- tiles 12-127: steady state, one tile per iteration
            pending = None
            for t in range(4 * JM_GROUPS, TILES):
                g = t // TPG
                chunk = chunk_of(g)
                if t % TPG == 0 and g + 3 < GROUPS and (g + 3) not in gbufs:
                    # prefetch three groups ahead at each group start
                    # (xp bufs=4: active + 3 in flight)
                    issue_group(g + 3)
                ps = ps_tiles[t]
                if t + 2 < TILES + 1 and (t + 1) > 12:
                    # bias write for the NEXT tile, one tile ahead of
                    # the PE so the in-order scalar queue never stalls
                    # the matmul stream (t=12's was emitted in the
                    # prologue block)
                    bias_write(t + 1)
                # deferred flush: emitted after the next tile's bias
                # copy so the token-DMA issue never parks ahead of an
                # x prefetch issue on the sync queue
                if pending is not None:
                    emit_flush(*pending)
                    pending = None

                for j in range(DCH):
                    nc.tensor.matmul(
                        ps[:],
                        lhsT=chunk(j, t % TPG),
                        rhs=ctiles[j][:],
                        start=False,
                        stop=(j == DCH - 1),
                        skip_group_check=True,
                    )
                dve_tile(t, ps[:])
                if (t + 1) in FLUSH:
                    s = FLUSH[FLUSH.index(t + 1) - 1] if (t + 1) != FLUSH[0] else 0
                    if t + 1 == TILES:
                        # final 1-tile store reads idxbuf directly
                        # (u32 index bits == i32 token), skipping the
                        # Pool CAST hop after the last match
                        nc.sync.dma_start(
                            out=out[:, t : t + 1],
                            in_=idxbuf[:, t, 0:1].bitcast(I32),
                        )
                    elif t + 1 == TILES - 1:
                        # the 7-tile store overlaps tile 127's argmax
                        emit_flush(s, t + 1)
                    else:
                        pending = (s, t + 1)

    _drop_implied_waits(nc)
    _hoist_excess_waits(nc)
    return nc


def _drop_implied_waits(nc: bass.Bass):
    """Drop sem waits that are transitively guaranteed.

    Engine queues are in-order, so when instruction I waits (sem_a >= n)
    and the updater engine of sem_a had itself waited (sem_b >= k) at or
    before the update that brings sem_a to n, then I's own (sem_b >= k')
    wait with k' <= k is redundant. This removes the per-tile hoisted-
    wait NOP on the PE queue (the first matmul of each tile waits both
    the bias-ACTIVATE sem and the DVE bank-WAR sem, but the bias
    ACTIVATE already waited that same WAR).
    Only applied to sem-ge-imm waits against sem-inc-imm updates of
    single-updater semaphores.
    """
    import bisect

    for f in nc.m.functions:
        for blk in f.blocks:
            insts = blk.instructions
            # sem -> updating engines
            upd_eng = {}
            for inst in insts:
                si = inst.sync_info
                if si and si.on_update:
                    for u in si.on_update:
                        upd_eng.setdefault(u.id, set()).add(inst.engine)
            single = {s for s, es in upd_eng.items() if len(es) == 1}

            # forward pass: per-engine waited-so-far map; per-sem list of
            # (cumulative_count_after_update, snapshot_of_waited)
            waited = {}
            hist = {}
            cum = {}
            for inst in insts:
                si = inst.sync_info
                if not si:
                    continue
                eng = inst.engine
                wmap = waited.setdefault(eng, {})
                if si.on_wait:
                    for w in si.on_wait:
                        if (
                            w.wait_mode == "sem-ge-imm"
                            and w.wait_value is not None
                        ):
                            if wmap.get(w.id, -1) < w.wait_value:
                                wmap[w.id] = w.wait_value
                if si.on_update:
                    for u in si.on_update:
                        if (
                            u.id in single
                            and u.update_mode == "sem-inc"
                            and u.update_reg is None
                            and u.update_value is not None
                        ):
                            c = cum.get(u.id, 0) + u.update_value
                            cum[u.id] = c
                            hist.setdefault(u.id, []).append((c, dict(wmap)))

            def guaranteed(sem_a, n, sem_b, k):
                h = hist.get(sem_a)
                if not h:
                    return False
                i = bisect.bisect_left(h, (n, ))
                if i >= len(h):
                    return False
                # snapshot at the FIRST update reaching >= n (weakest)
                return h[i][1].get(sem_b, -1) >= k

            # drop pass — PE matmuls only: the analyzed-safe case is
            # the tile's first matmul waiting both the bias-ACTIVATE
            # sem and the DVE bank-WAR sem the ACTIVATE already waited
            # (an unrestricted version raced in the token path)
            for inst in insts:
                si = inst.sync_info
                if not si or not si.on_wait or len(si.on_wait) < 2:
                    continue
                if inst.engine != mybir.EngineType.PE or not isinstance(
                    inst, mybir.InstMatmult
                ):
                    continue
                ws = list(si.on_wait)
                keep = list(ws)
                for wb in ws:
                    if wb.wait_mode != "sem-ge-imm" or wb.wait_value is None:
                        continue
                    for wa in keep:
                        if wa is wb or wa.wait_mode != "sem-ge-imm":
                            continue
                        if wa.wait_value is None:
                            continue
                        if guaranteed(wa.id, wa.wait_value, wb.id, wb.wait_value):
                            keep = [w for w in keep if w is not wb]
                            break
                if len(keep) != len(ws):
                    si.on_wait = keep
                    inst.sync_info = si


def _hoist_excess_waits(nc: bass.Bass, max_waits: int = 1):
    n = 0
    for f in nc.m.functions:
        for blk in f.blocks:
            insts = blk.instructions
            i = 0
            while i < len(insts):
                inst = insts[i]
                si = inst.sync_info
                if si and si.on_wait and len(si.on_wait) > max_waits:
                    waits = list(si.on_wait)
                    si.on_wait = waits[-max_waits:]
                    inst.sync_info = si
                    pre = []
                    for j in range(0, len(waits) - max_waits, max_waits):
                        nd = mybir.InstNoOp(name=f"I-wsplit{n}", ins=[], outs=[])
                        n += 1
                        nd.engine = inst.engine
                        nsi = type(si)(
                            on_wait=waits[j : j + max_waits], on_update=[]
                        )
                        nd.sync_info = nsi
                        try:
                            nc.register_instruction(nd, overwrite=True)
                        except Exception:
                            pass
                        pre.append(nd)
                    for k, nd in enumerate(pre):
                        insts.insert(i + k, nd)
                    i += len(pre)
                i += 1


def make_in_maps(x, C, Cnorm):
    x16 = x.astype(np.float16)
    C16 = C.astype(np.float16).reshape(DCH, P, K)

    bz = (-0.5 * Cnorm.reshape(K)).astype(np.float32)
    bh = bz.astype(np.float16)
    bl = (bz - bh.astype(np.float32)).astype(np.float16)

    cons = np.zeros((P, DCH * K + K + P), np.float16)
    cons[0, :K] = bh
    cons[1, :K] = bl
    cons[0:2, K : K + P] = 1.0
    cons[:, K + P :] = C16.transpose(1, 0, 2).reshape(P, DCH * K)

    in_maps = []
    for c in range(NCORES):
        xs = x16[c * ROWS : (c + 1) * ROWS]
        xr = xs.reshape(P, GROUPS, TPG, DCH, P)
        xgc = np.ascontiguousarray(xr.transpose(1, 4, 3, 2, 0))
        in_maps.append(
            {
                "xg": xgc.reshape(GROUPS, P, DCH * TPG * P),
                "cons": cons,
            }
        )
    return in_maps


_NC_CACHE = {}


def kernel(x, C, Cnorm, b, t):
    global LAST_RESULT
    x = np.asarray(x)
    C = np.asarray(C)
    Cnorm = np.asarray(Cnorm)

    key = 0
    if key not in _NC_CACHE:
        _NC_CACHE[key] = build_nc()
    nc = _NC_CACHE[key]

    in_maps = make_in_maps(x, C, Cnorm)
    trace = bool(int(os.environ.get("KM_TRACE", "0")))
    if trace:
        _ensure_ntff_hook()
    res = run_bass_kernel_spmd(
        nc, in_maps, core_ids=list(range(NCORES)), trace=trace
    )
    LAST_RESULT = res

    shards = [res.results[c]["out"].reshape(-1) for c in range(NCORES)]
    tokens = np.concatenate(shards).astype(np.int32)
    return tokens.reshape(int(b), int(t))
